# revision 6
# baseline (speedup 1.0000x reference)
"""CNF forward (vector field + exact Jacobian trace) on 8 TRN2 cores.

Math: reference computes, per sample x (row of state[:, 1:]):
    f(x)  = W3^T tanh(W2^T tanh(W1^T [x; t] + b1) + b2) + b3      (dx)
    trJ   = trace(df/dx)                                          (aug = -trJ)

Instead of D=64 JVPs per sample, use the closed form of the trace:
    h1 = tanh([x;t] @ W1 + b1),  h2 = tanh(h1 @ W2 + b2)
    s1 = 1 - h1^2,               s2 = 1 - h2^2
    trJ = s1^T F s2   with  F[h',h] = W2[h',h] * (W3 @ W1[:D])[h, h']
F depends only on the weights and is computed once on-device per core
(cheap: one K=64 matmul + one elementwise multiply).

Sharding: data-parallel. Batch B=1024 split as 128 samples per core;
weights replicated. Layout on-chip is feature-major (batch on the free
dim), so W1/W2/W3 are used directly as matmul lhsT operands with no
weight transposes; only the [128, 65] state tile is PE-transposed.
"""

import numpy as np

import concourse.bass as bass
import concourse.bacc as bacc
import concourse.tile as tile
from concourse import mybir
from concourse.bass_utils import run_bass_kernel_spmd
from concourse.masks import make_identity

B, D, H = 1024, 64, 512
NCORES = 8
BC = B // NCORES  # 128 samples per core
KT = H // 128     # 4 feature tiles of 128
F32 = mybir.dt.float32
AF = mybir.ActivationFunctionType
ALU = mybir.AluOpType
ts = bass.ts

_NC = None


def _build():
    # Bacc (not plain Bass): its compile() pipeline legalizes sync waits —
    # matmuls can carry only one HW wait slot; extra waits move to
    # ldweights / event-semaphore instructions.
    nc = bacc.Bacc()

    st = nc.declare_dram_parameter("state", [BC, D + 1], F32, isOutput=False)
    t_in = nc.declare_dram_parameter("t", [1, 1], F32, isOutput=False)
    W1 = nc.declare_dram_parameter("W1", [D + 1, H], F32, isOutput=False)
    b1 = nc.declare_dram_parameter("b1", [H, 1], F32, isOutput=False)
    W2 = nc.declare_dram_parameter("W2", [H, H], F32, isOutput=False)
    b2 = nc.declare_dram_parameter("b2", [H, 1], F32, isOutput=False)
    W3 = nc.declare_dram_parameter("W3", [H, D], F32, isOutput=False)
    b3 = nc.declare_dram_parameter("b3", [D, 1], F32, isOutput=False)
    out = nc.declare_dram_parameter("out", [BC, D + 1], F32, isOutput=True)

    with tile.TileContext(nc) as tc:
        with (
            tc.tile_pool(name="const", bufs=1) as cp,
            tc.tile_pool(name="act", bufs=1) as ap,
            tc.tile_pool(name="tmp", bufs=2) as tp,
            tc.tile_pool(name="ps", bufs=1, space="PSUM") as ps,
        ):
            # ---------------- constants / weights ----------------
            ident = cp.tile([128, 128], F32, tag="ident")
            make_identity(nc, ident)
            ones = cp.tile([128, 1], F32, tag="ones")
            nc.vector.memset(ones, 1.0)
            t_sb = cp.tile([1, 1], F32, tag="t_sb")
            nc.sync.dma_start(out=t_sb, in_=t_in[:, :])

            # W1 with rows rotated so row 0 is the t-row: matches stateT
            # whose row 0 (the logp channel, unused) is overwritten with t.
            w1_sb = cp.tile([D + 1, H], F32, tag="w1")
            nc.sync.dma_start(out=w1_sb[0:1, :], in_=W1[D:D + 1, :])
            nc.sync.dma_start(out=w1_sb[1:D + 1, :], in_=W1[0:D, :])
            # x-rows of W1 at base partition 0 (matmul lhsT requires base 0)
            w1x_sb = cp.tile([D, H], F32, tag="w1x")
            nc.sync.dma_start(out=w1x_sb, in_=W1[0:D, :])

            w2_sb = []
            for k in range(KT):
                w2k = cp.tile([128, H], F32, tag=f"w2_{k}")
                nc.sync.dma_start(out=w2k, in_=W2[ts(k, 128), :])
                w2_sb.append(w2k)
            w3_sb = []
            for k in range(KT):
                w3k = cp.tile([128, D], F32, tag=f"w3_{k}")
                nc.sync.dma_start(out=w3k, in_=W3[ts(k, 128), :])
                w3_sb.append(w3k)

            b1_sb, b2_sb = [], []
            for k in range(KT):
                b1k = cp.tile([128, 1], F32, tag=f"b1_{k}")
                nc.sync.dma_start(out=b1k, in_=b1[ts(k, 128), :])
                b1_sb.append(b1k)
                b2k = cp.tile([128, 1], F32, tag=f"b2_{k}")
                nc.sync.dma_start(out=b2k, in_=b2[ts(k, 128), :])
                b2_sb.append(b2k)
            b3_sb = cp.tile([D, 1], F32, tag="b3")
            nc.sync.dma_start(out=b3_sb, in_=b3[:, :])

            # ---------------- state -> stateT (feature-major) ----------------
            state_sb = ap.tile([BC, D + 1], F32, tag="state")
            nc.sync.dma_start(out=state_sb, in_=st[:, :])
            stT_ps = ps.tile([D + 1, BC], F32, tag="tp", bufs=2)
            nc.tensor.transpose(stT_ps, state_sb, ident)
            stT = ap.tile([D + 1, BC], F32, tag="stT")
            nc.scalar.copy(stT, stT_ps)
            # row 0 (logp channel) := t
            nc.vector.tensor_scalar(stT[0:1, :], stT[0:1, :], 0.0, t_sb,
                                    ALU.mult, ALU.add)

            # ---------------- layer 1: h1T, s1T ----------------
            h1, s1 = [], []
            for j in range(KT):
                z1_ps = ps.tile([128, BC], F32, tag="z", bufs=2)
                nc.tensor.matmul(z1_ps, w1_sb[:, ts(j, 128)], stT,
                                 start=True, stop=True)
                h = ap.tile([128, BC], F32, tag=f"h1_{j}")
                nc.scalar.activation(h, z1_ps, AF.Tanh, bias=b1_sb[j])
                sq = tp.tile([128, BC], F32, tag="sq")
                nc.vector.tensor_mul(sq, h, h)
                s = ap.tile([128, BC], F32, tag=f"s1_{j}")
                nc.vector.tensor_scalar(s, sq, -1.0, 1.0, ALU.mult, ALU.add)
                h1.append(h)
                s1.append(s)

            # ---------------- layer 2: h2T, s2T ----------------
            h2, s2 = [], []
            for j in range(KT):
                z2_ps = ps.tile([128, BC], F32, tag="z", bufs=2)
                for k in range(KT):
                    nc.tensor.matmul(z2_ps, w2_sb[k][:, ts(j, 128)], h1[k],
                                     start=(k == 0), stop=(k == KT - 1))
                h = ap.tile([128, BC], F32, tag=f"h2_{j}")
                nc.scalar.activation(h, z2_ps, AF.Tanh, bias=b2_sb[j])
                sq = tp.tile([128, BC], F32, tag="sq")
                nc.vector.tensor_mul(sq, h, h)
                s = ap.tile([128, BC], F32, tag=f"s2_{j}")
                nc.vector.tensor_scalar(s, sq, -1.0, 1.0, ALU.mult, ALU.add)
                h2.append(h)
                s2.append(s)

            # ---------------- layer 3: outT = W3^T h2T (+ b3) ----------------
            o_ps = ps.tile([D, BC], F32, tag="oacc", bufs=1)
            for k in range(KT):
                nc.tensor.matmul(o_ps, w3_sb[k], h2[k],
                                 start=(k == 0), stop=(k == KT - 1))
            outT = ap.tile([D, BC], F32, tag="outT")
            nc.vector.tensor_scalar_add(outT, o_ps, b3_sb)

            # ---------------- trace weight matrix F ----------------
            # W3T[a, h] = W3[h, a] via PE transpose, tile by tile
            w3T = ap.tile([D, H], F32, tag="w3T")
            for k in range(KT):
                w3T_ps = ps.tile([D, 128], F32, tag="tp", bufs=2)
                nc.tensor.transpose(w3T_ps, w3_sb[k], ident)
                nc.scalar.copy(w3T[:, ts(k, 128)], w3T_ps)

            # E2T[h',h] = sum_a W1[a,h'] W3[h,a];  F = W2 * E2T (elementwise)
            f_sb = []
            for m in range(KT):
                e2t_ps = ps.tile([128, H], F32, tag="e2t", bufs=2)
                nc.tensor.matmul(e2t_ps, w1x_sb[:, ts(m, 128)], w3T,
                                 start=True, stop=True)
                fm = ap.tile([128, H], F32, tag=f"f_{m}")
                nc.vector.tensor_mul(fm, w2_sb[m], e2t_ps)
                f_sb.append(fm)

            # ---------------- trJ = s1^T F s2, batched ----------------
            # T2[h, i] = sum_h' F[h', h] * s1T[h', i]
            tr_ps = ps.tile([BC, 1], F32, tag="tracc", bufs=1)
            for m in range(KT):
                t2_ps = ps.tile([128, BC], F32, tag="z", bufs=2)
                for k in range(KT):
                    nc.tensor.matmul(t2_ps, f_sb[k][:, ts(m, 128)], s1[k],
                                     start=(k == 0), stop=(k == KT - 1))
                pm = tp.tile([128, BC], F32, tag="pm")
                nc.vector.tensor_mul(pm, t2_ps, s2[m])
                # tr[i] += sum_h pm[h, i]  (partition reduce via ones matmul)
                nc.tensor.matmul(tr_ps, pm, ones,
                                 start=(m == 0), stop=(m == KT - 1))

            # ---------------- assemble [aug | dx] and store ----------------
            final_sb = ap.tile([BC, D + 1], F32, tag="final")
            nc.scalar.mul(final_sb[:, 0:1], tr_ps, -1.0)  # aug = -trJ
            oT_ps = ps.tile([BC, D], F32, tag="tp", bufs=2)
            nc.tensor.transpose(oT_ps, outT, ident[0:D, 0:D])
            nc.scalar.copy(final_sb[:, 1:D + 1], oT_ps)
            nc.sync.dma_start(out=out[:, :], in_=final_sb)

    nc.finalize()
    return nc


def _get_nc():
    global _NC
    if _NC is None:
        _NC = _build()
    return _NC


def kernel(**inputs) -> np.ndarray:
    f32 = lambda a: np.ascontiguousarray(np.asarray(a), dtype=np.float32)
    state = f32(inputs["state"])
    t = f32(inputs["t"]).reshape(1, 1)
    W1 = f32(inputs["W1"])
    b1 = f32(inputs["b1"]).reshape(H, 1)
    W2 = f32(inputs["W2"])
    b2 = f32(inputs["b2"]).reshape(H, 1)
    W3 = f32(inputs["W3"])
    b3 = f32(inputs["b3"]).reshape(D, 1)

    in_maps = []
    for c in range(NCORES):
        in_maps.append({
            "state": np.ascontiguousarray(state[c * BC:(c + 1) * BC]),
            "t": t, "W1": W1, "b1": b1, "W2": W2, "b2": b2,
            "W3": W3, "b3": b3,
        })

    res = run_bass_kernel_spmd(_get_nc(), in_maps, list(range(NCORES))).results
    return np.concatenate([res[c]["out"] for c in range(NCORES)], axis=0)


# revision 11
# speedup vs baseline: 1.3272x; 1.3272x over previous
"""CNF forward (vector field + exact Jacobian trace) on 8 TRN2 cores.

Math: reference computes, per sample x (row of state[:, 1:]):
    f(x)  = W3^T tanh(W2^T tanh(W1^T [x; t] + b1) + b2) + b3      (dx)
    trJ   = trace(df/dx)                                          (aug = -trJ)

Instead of D=64 JVPs per sample, use the closed form of the trace:
    h1 = tanh([x;t] @ W1 + b1),  h2 = tanh(h1 @ W2 + b2)
    s1 = 1 - h1^2,               s2 = 1 - h2^2
    trJ = s1^T F s2   with  F[h',h] = W2[h',h] * (W3 @ W1[:D])[h, h']
F depends only on the weights and is computed on-device per core
(one K=64 matmul + one elementwise multiply per 128-row tile).

Sharding: data-parallel, 128 samples per core, weights replicated.

Layout: layer 1 runs feature-major (h1T tiles [128h x 128b]) so W1 is
lhsT directly; layers 2/3 and the trace matmul run batch-major with the
feature-major activations as lhsT — every matmul has N=512 or N=128 and
there are no weight transposes (only state, W3, and h2 get PE-transposed).
The trace contraction is a single fused DVE tensor_tensor_reduce.
"""

import numpy as np

import concourse.bacc as bacc
import concourse.bass as bass
import concourse.tile as tile
from concourse import mybir
from concourse.bass_utils import run_bass_kernel_spmd
from concourse.masks import make_identity

B, D, H = 1024, 64, 512
NCORES = 8
BC = B // NCORES  # 128 samples per core
KT = H // 128     # 4 feature tiles of 128
F32 = mybir.dt.float32
AF = mybir.ActivationFunctionType
ALU = mybir.AluOpType
ts = bass.ts

_NC = {}

# hardware-bisect toggles (TTR crashes the device - NRT_EXEC_UNIT_UNRECOVERABLE)
USE_SCALAR_DMA = True   # issue some loads on the Activation HWDGE queue
USE_GPSIMD_ELT = True   # compute s1/s2 on GpSimd instead of DVE
USE_TTR = False         # fused tensor_tensor_reduce: hangs/crashes on HW


def _build(with_bias23: bool):
    """with_bias23: include the rank-1 bias adds for b2/b3 (batch-major
    layers can't take a per-free-dim bias via ACT). setup_inputs() has
    zero biases, so the fast path skips them; correctness for nonzero
    b2/b3 is preserved via this variant."""
    nc = bacc.Bacc()
    dma2 = nc.scalar if USE_SCALAR_DMA else nc.sync

    st = nc.declare_dram_parameter("state", [BC, D + 1], F32, isOutput=False)
    W1 = nc.declare_dram_parameter("W1", [D + 1, H], F32, isOutput=False)
    W2 = nc.declare_dram_parameter("W2", [H, H], F32, isOutput=False)
    W3 = nc.declare_dram_parameter("W3", [H, D], F32, isOutput=False)
    # packed per-partition constants: cols 0-3 b1 tiles, col 4 t (bcast),
    # cols 5-6: b2/b3 rows are only used by the bias variant
    cblk = nc.declare_dram_parameter("cblk", [128, 5], F32, isOutput=False)
    if with_bias23:
        b2r = nc.declare_dram_parameter("b2r", [1, H], F32, isOutput=False)
        b3r = nc.declare_dram_parameter("b3r", [1, D], F32, isOutput=False)
    out = nc.declare_dram_parameter("out", [BC, D + 1], F32, isOutput=True)

    with tile.TileContext(nc) as tc:
        with (
            tc.tile_pool(name="const", bufs=1) as cp,
            tc.tile_pool(name="act", bufs=1) as ap,
            tc.tile_pool(name="ps", bufs=1, space="PSUM") as ps,
        ):
            # ---------------- loads: critical path first ----------------
            # scalar (Activation HWDGE queue): cblk, state, then F-path
            cblk_sb = cp.tile([128, 5], F32, tag="cblk")
            dma2.dma_start(out=cblk_sb, in_=cblk[:, :])
            state_sb = ap.tile([BC, D + 1], F32, tag="state")
            dma2.dma_start(out=state_sb, in_=st[:, :])
            # sync (SP HWDGE queue): W1 (rotated), then W2 k-tiles
            w1r = cp.tile([D + 1, H], F32, tag="w1r")
            nc.sync.dma_start(out=w1r[0:1, :], in_=W1[D:D + 1, :])
            nc.sync.dma_start(out=w1r[1:D + 1, :], in_=W1[0:D, :])
            w3_sb = []
            for k in range(KT):
                w3k = cp.tile([128, D], F32, tag=f"w3_{k}")
                dma2.dma_start(out=w3k, in_=W3[ts(k, 128), :])
                w3_sb.append(w3k)
            w1x = cp.tile([D, H], F32, tag="w1x")
            dma2.dma_start(out=w1x, in_=W1[0:D, :])
            w2_sb = []
            for k in range(KT):
                w2k = cp.tile([128, H], F32, tag=f"w2_{k}")
                nc.sync.dma_start(out=w2k, in_=W2[ts(k, 128), :])
                w2_sb.append(w2k)
            if with_bias23:
                b2r_sb = cp.tile([1, H], F32, tag="b2r")
                nc.sync.dma_start(out=b2r_sb, in_=b2r[:, :])
                b3r_sb = cp.tile([1, D], F32, tag="b3r")
                nc.sync.dma_start(out=b3r_sb, in_=b3r[:, :])
                onesr = cp.tile([1, BC], F32, tag="onesr")
                nc.vector.memset(onesr, 1.0)

            ident = cp.tile([128, 128], F32, tag="ident")
            make_identity(nc, ident)

            # ---------------- state -> stT (feature-major) ----------------
            stT_ps = ps.tile([D + 1, BC], F32, tag="tp", bufs=2)
            nc.tensor.transpose(stT_ps, state_sb, ident)
            stT = ap.tile([D + 1, BC], F32, tag="stT")
            nc.scalar.copy(stT, stT_ps)
            # row 0 (logp channel, unused) := t  (w1r row 0 is the t-row)
            nc.vector.tensor_scalar(stT[0:1, :], stT[0:1, :], 0.0,
                                    cblk_sb[0:1, 4:5], ALU.mult, ALU.add)

            # ---------------- layer 1 (feature-major): h1T, s1T ----------
            h1, s1 = [], []
            for j in range(KT):
                z1_ps = ps.tile([128, BC], F32, tag="tp", bufs=2)
                nc.tensor.matmul(z1_ps, w1r[:, ts(j, 128)], stT,
                                 start=True, stop=True)
                h = ap.tile([128, BC], F32, tag=f"h1_{j}")
                nc.scalar.activation(h, z1_ps, AF.Tanh,
                                     bias=cblk_sb[:, j:j + 1])
                s = ap.tile([128, BC], F32, tag=f"s1_{j}")
                elt = nc.gpsimd if USE_GPSIMD_ELT else nc.vector
                elt.tensor_mul(s, h, h)
                elt.tensor_scalar(s, s, -1.0, 1.0, ALU.mult, ALU.add)
                h1.append(h)
                s1.append(s)

            # ---------------- trace weight matrix F ----------------------
            w3T = ap.tile([D, H], F32, tag="w3T")
            for k in range(KT):
                w3T_ps = ps.tile([D, BC], F32, tag="tp", bufs=2)
                nc.tensor.transpose(w3T_ps, w3_sb[k], ident)
                nc.vector.tensor_copy(w3T[:, ts(k, 128)], w3T_ps)
            f_sb = []
            for m in range(KT):
                e2t_ps = ps.tile([128, H], F32, tag="e2t", bufs=2)
                nc.tensor.matmul(e2t_ps, w1x[:, ts(m, 128)], w3T,
                                 start=True, stop=True)
                fm = ap.tile([128, H], F32, tag=f"f_{m}")
                nc.vector.tensor_mul(fm, w2_sb[m], e2t_ps)
                f_sb.append(fm)

            # ---------------- layer 2 (batch-major): h2, s2 --------------
            z2_ps = ps.tile([BC, H], F32, tag="z2", bufs=1)
            for k in range(KT):
                nc.tensor.matmul(z2_ps, h1[k], w2_sb[k],
                                 start=(k == 0), stop=(k == KT - 1 and not with_bias23))
            if with_bias23:
                nc.tensor.matmul(z2_ps, onesr, b2r_sb, start=False, stop=True)
            h2 = ap.tile([BC, H], F32, tag="h2")
            h2T = []
            for j in range(KT):
                nc.scalar.activation(h2[:, ts(j, 128)], z2_ps[:, ts(j, 128)],
                                     AF.Tanh)
                hT_ps = ps.tile([128, BC], F32, tag="tp", bufs=2)
                nc.tensor.transpose(hT_ps, h2[:, ts(j, 128)], ident)
                hT = ap.tile([128, BC], F32, tag=f"h2T_{j}")
                nc.vector.tensor_copy(hT, hT_ps)
                h2T.append(hT)
            s2 = ap.tile([BC, H], F32, tag="s2")
            elt = nc.gpsimd if USE_GPSIMD_ELT else nc.vector
            elt.tensor_mul(s2, h2, h2)
            elt.tensor_scalar(s2, s2, -1.0, 1.0, ALU.mult, ALU.add)

            # ---------------- layer 3 (batch-major): dx ------------------
            o_ps = ps.tile([BC, D], F32, tag="o", bufs=1)
            for k in range(KT):
                nc.tensor.matmul(o_ps, h2T[k], w3_sb[k],
                                 start=(k == 0), stop=(k == KT - 1 and not with_bias23))
            if with_bias23:
                nc.tensor.matmul(o_ps, onesr, b3r_sb, start=False, stop=True)

            # ---------------- trJ = s1^T F s2 (batch-major) --------------
            t2_ps = ps.tile([BC, H], F32, tag="t2", bufs=1)
            for k in range(KT):
                nc.tensor.matmul(t2_ps, s1[k], f_sb[k],
                                 start=(k == 0), stop=(k == KT - 1))

            final_sb = ap.tile([BC, D + 1], F32, tag="final")
            ttr_scr = ap.tile([BC, H], F32, tag="ttr_scr")
            if USE_TTR:
                # aug = -sum_h T2*S2  (scale=-1 before the add-reduce)
                nc.vector.tensor_tensor_reduce(
                    out=ttr_scr, in0=t2_ps, in1=s2, scale=-1.0, scalar=0.0,
                    op0=ALU.mult, op1=ALU.add, accum_out=final_sb[:, 0:1])
            else:
                nc.vector.tensor_mul(ttr_scr, t2_ps, s2)
                nc.vector.tensor_reduce(
                    out=final_sb[:, 0:1], in_=ttr_scr,
                    op=ALU.add, axis=mybir.AxisListType.X, negate=True)
            nc.scalar.copy(final_sb[:, 1:D + 1], o_ps)
            nc.sync.dma_start(out=out[:, :], in_=final_sb)

    nc.finalize()
    return nc


def _get_nc(with_bias23: bool):
    key = bool(with_bias23)
    if key not in _NC:
        _NC[key] = _build(key)
    return _NC[key]


def make_in_maps(inputs):
    f32 = lambda a: np.ascontiguousarray(np.asarray(a), dtype=np.float32)
    state = f32(inputs["state"])
    t = f32(inputs["t"]).reshape(-1)[0]
    W1 = f32(inputs["W1"])
    b1 = f32(inputs["b1"]).reshape(H)
    W2 = f32(inputs["W2"])
    b2 = f32(inputs["b2"]).reshape(H)
    W3 = f32(inputs["W3"])
    b3 = f32(inputs["b3"]).reshape(D)

    with_bias23 = bool(np.any(b2) or np.any(b3))

    cb = np.empty((128, 5), np.float32)
    cb[:, 0:4] = b1.reshape(4, 128).T
    cb[:, 4] = t

    base = {"W1": W1, "W2": W2, "W3": W3, "cblk": cb}
    if with_bias23:
        base["b2r"] = b2.reshape(1, H)
        base["b3r"] = b3.reshape(1, D)
    in_maps = []
    for c in range(NCORES):
        m = dict(base)
        m["state"] = np.ascontiguousarray(state[c * BC:(c + 1) * BC])
        in_maps.append(m)
    return with_bias23, in_maps


def kernel(**inputs) -> np.ndarray:
    with_bias23, in_maps = make_in_maps(inputs)
    res = run_bass_kernel_spmd(_get_nc(with_bias23), in_maps,
                               list(range(NCORES))).results
    return np.concatenate([res[c]["out"] for c in range(NCORES)], axis=0)
